# revision 1
# baseline (speedup 1.0000x reference)
import sys

sys.path.insert(0, "/opt/trn_rl_repo")

import numpy as np

# Problem constants (hardcoded per spec nn_AdaptivePriorBoxesLoss)
P_TOT = 131072
T = 256
NCORES = 8
PL = P_TOT // NCORES  # 16384 priors per core
ROWS = 128
NT = PL // ROWS  # 128 tiles per core; local prior p = r*NT + j? -> p = r*128 + j
TH_F = 2.0 / 7.0  # iou > 0.4  <=>  inter/(areaA+areaB) > 2/7
K_VAL = 2.5
BETA = 1.0

_CACHE = {}


def _build(phase1_reps=1):
    from concourse import bass, mybir, tile
    from concourse.masks import make_identity

    f32 = mybir.dt.float32
    i32 = mybir.dt.int32
    u32 = mybir.dt.uint32
    AF = mybir.ActivationFunctionType
    OP = mybir.AluOpType

    nc = bass.Bass()
    locs_ext = nc.declare_dram_parameter("locs", [PL, 2], f32, isOutput=False)
    params_ext = nc.declare_dram_parameter("params", [PL, 3], f32, isOutput=False)
    truths_ext = nc.declare_dram_parameter("truths", [T, 4], f32, isOutput=False)
    poff_ext = nc.declare_dram_parameter("poff", [1, 1], f32, isOutput=False)
    out_ext = nc.declare_dram_parameter("out", [1, 1], f32, isOutput=True)

    RG = [list(range(NCORES))]

    with tile.TileContext(nc) as tc:
        with (
            tc.tile_pool(name="persist", bufs=1) as pe,
            tc.tile_pool(name="work", bufs=3) as wp,
            tc.tile_pool(name="psum", bufs=2, space="PSUM") as pp,
            tc.tile_pool(name="dram", bufs=1, space="DRAM") as dp,
        ):
            # ---------------- Phase 0: prep ----------------
            locs_sb = pe.tile([ROWS, NT, 2], f32, name="locs_sb")
            params_sb = pe.tile([ROWS, NT, 3], f32, name="params_sb")
            nc.sync.dma_start(
                out=locs_sb[:], in_=locs_ext[:].rearrange("(r j) c -> r j c", r=ROWS)
            )
            nc.sync.dma_start(
                out=params_sb[:],
                in_=params_ext[:].rearrange("(r j) c -> r j c", r=ROWS),
            )


            # PE broadcast helper: [1, n] row -> [ROWS, n]
            ones128 = pe.tile([ROWS, 1], f32, name="ones128")
            nc.vector.memset(ones128[:], 1.0)
            ident = pe.tile([ROWS, ROWS], f32, name="ident")
            make_identity(nc, ident[:])

            # DMA partition-broadcast of raw inputs (single-dep DMAs), then
            # derive the computed truth tiles directly in broadcast form.
            def bcast_dram(src_ap, name, n=T):
                tmp = pe.tile([ROWS, n], f32, name="tmp_" + name)
                nc.sync.dma_start(out=tmp[:], in_=src_ap.to_broadcast([ROWS, n]))
                sb = pe.tile([ROWS, n], f32, name=name)
                nc.scalar.copy(out=sb[:], in_=tmp[:])
                return sb

            TX1b = bcast_dram(truths_ext[None, :, 0], "TX1b")
            TY1b = bcast_dram(truths_ext[None, :, 1], "TY1b")
            TX2b = bcast_dram(truths_ext[None, :, 2], "TX2b")
            TY2b = bcast_dram(truths_ext[None, :, 3], "TY2b")
            POFFb = bcast_dram(poff_ext[:], "POFFb", n=1)

            TWb = pe.tile([ROWS, T], f32, name="TWb")
            THb = pe.tile([ROWS, T], f32, name="THb")
            TAb = pe.tile([ROWS, T], f32, name="TAb")
            TCX10b = pe.tile([ROWS, T], f32, name="TCX10b")
            TCY10b = pe.tile([ROWS, T], f32, name="TCY10b")
            LTW5b = pe.tile([ROWS, T], f32, name="LTW5b")
            LTH5b = pe.tile([ROWS, T], f32, name="LTH5b")
            nc.vector.tensor_tensor(out=TWb[:], in0=TX2b[:], in1=TX1b[:], op=OP.subtract)
            nc.vector.tensor_tensor(out=THb[:], in0=TY2b[:], in1=TY1b[:], op=OP.subtract)
            nc.vector.tensor_tensor(out=TAb[:], in0=TWb[:], in1=THb[:], op=OP.mult)
            nc.vector.tensor_tensor(out=TCX10b[:], in0=TX1b[:], in1=TX2b[:], op=OP.add)
            nc.vector.tensor_scalar(
                out=TCX10b[:], in0=TCX10b[:], scalar1=5.0, scalar2=None, op0=OP.mult
            )
            nc.vector.tensor_tensor(out=TCY10b[:], in0=TY1b[:], in1=TY2b[:], op=OP.add)
            nc.vector.tensor_scalar(
                out=TCY10b[:], in0=TCY10b[:], scalar1=5.0, scalar2=None, op0=OP.mult
            )
            nc.scalar.activation(out=LTW5b[:], in_=TWb[:], func=AF.Ln)
            nc.vector.tensor_scalar(
                out=LTW5b[:], in0=LTW5b[:], scalar1=5.0, scalar2=None, op0=OP.mult
            )
            nc.scalar.activation(out=LTH5b[:], in_=THb[:], func=AF.Ln)
            nc.vector.tensor_scalar(
                out=LTH5b[:], in0=LTH5b[:], scalar1=5.0, scalar2=None, op0=OP.mult
            )

            # iotas
            it_i = pe.tile([ROWS, T], i32, name="it_i")
            nc.gpsimd.iota(it_i[:], [[1, T]], base=0, channel_multiplier=0)
            IOTA_TF = pe.tile([ROWS, T], f32, name="IOTA_TF")
            nc.vector.tensor_copy(out=IOTA_TF[:], in_=it_i[:])
            ip_i = pe.tile([ROWS, T], i32, name="ip_i")
            nc.gpsimd.iota(ip_i[:], [[0, T]], base=0, channel_multiplier=NT)
            IOTA_PRI = pe.tile([ROWS, T], f32, name="IOTA_PRI")
            nc.vector.tensor_copy(out=IOTA_PRI[:], in_=ip_i[:])
            rid_i = pe.tile([ROWS, 1], i32, name="rid_i")
            nc.gpsimd.iota(rid_i[:], [[0, 1]], base=0, channel_multiplier=1)
            ridf = pe.tile([ROWS, 1], f32, name="ridf")
            nc.vector.tensor_copy(out=ridf[:], in_=rid_i[:])
            off8_i = pe.tile([ROWS, 8], i32, name="off8_i")
            nc.gpsimd.iota(off8_i[:], [[PL, 8]], base=0, channel_multiplier=0)
            OFF8 = pe.tile([ROWS, 8], f32, name="OFF8")
            nc.vector.tensor_copy(out=OFF8[:], in_=off8_i[:])

            # per-prior derived arrays [ROWS, NT]
            locs_l = pe.tile([ROWS, NT, 2], f32, name="locs_l")
            nc.scalar.copy(out=locs_l[:], in_=locs_sb[:])
            params_l = pe.tile([ROWS, NT, 3], f32, name="params_l")
            nc.scalar.copy(out=params_l[:], in_=params_sb[:])
            cxv = locs_l[:, :, 0]
            cyv = locs_l[:, :, 1]
            wv = params_l[:, :, 0]
            hv = params_l[:, :, 1]
            av = params_l[:, :, 2]

            halfw = pe.tile([ROWS, NT], f32, name="halfw")
            halfh = pe.tile([ROWS, NT], f32, name="halfh")
            px1 = pe.tile([ROWS, NT], f32, name="px1")
            px2 = pe.tile([ROWS, NT], f32, name="px2")
            py1 = pe.tile([ROWS, NT], f32, name="py1")
            py2 = pe.tile([ROWS, NT], f32, name="py2")
            areap = pe.tile([ROWS, NT], f32, name="areap")
            pcx10 = pe.tile([ROWS, NT], f32, name="pcx10")
            pcy10 = pe.tile([ROWS, NT], f32, name="pcy10")
            rpw = pe.tile([ROWS, NT], f32, name="rpw")
            rph = pe.tile([ROWS, NT], f32, name="rph")
            lpw5 = pe.tile([ROWS, NT], f32, name="lpw5")
            lph5 = pe.tile([ROWS, NT], f32, name="lph5")
            s_sb = pe.tile([ROWS, NT], f32, name="s_sb")

            nc.vector.tensor_scalar(
                out=halfw[:], in0=wv, scalar1=0.5, scalar2=None, op0=OP.mult
            )
            nc.vector.tensor_scalar(
                out=halfh[:], in0=hv, scalar1=0.5, scalar2=None, op0=OP.mult
            )
            nc.vector.tensor_tensor(out=px1[:], in0=cxv, in1=halfw[:], op=OP.subtract)
            nc.vector.tensor_tensor(out=px2[:], in0=cxv, in1=halfw[:], op=OP.add)
            nc.vector.tensor_tensor(out=py1[:], in0=cyv, in1=halfh[:], op=OP.subtract)
            nc.vector.tensor_tensor(out=py2[:], in0=cyv, in1=halfh[:], op=OP.add)
            nc.vector.tensor_tensor(out=areap[:], in0=wv, in1=hv, op=OP.mult)
            nc.vector.tensor_scalar(
                out=pcx10[:], in0=cxv, scalar1=10.0, scalar2=None, op0=OP.mult
            )
            nc.vector.tensor_scalar(
                out=pcy10[:], in0=cyv, scalar1=10.0, scalar2=None, op0=OP.mult
            )
            nc.vector.reciprocal(out=rpw[:], in_=wv)
            nc.vector.reciprocal(out=rph[:], in_=hv)
            nc.scalar.activation(out=lpw5[:], in_=wv, func=AF.Ln)
            nc.vector.tensor_scalar(
                out=lpw5[:], in0=lpw5[:], scalar1=5.0, scalar2=None, op0=OP.mult
            )
            nc.scalar.activation(out=lph5[:], in_=hv, func=AF.Ln)
            nc.vector.tensor_scalar(
                out=lph5[:], in0=lph5[:], scalar1=5.0, scalar2=None, op0=OP.mult
            )
            nc.scalar.activation(out=s_sb[:], in_=av, func=AF.Sigmoid)

            # payload tables PAY_g[t, c] for c in (tcx10, tcy10, ltw5, lth5),
            # t split in two 128-chunks; used by per-tile PE gather matmuls
            PAYg = []
            for g in range(2):
                gsl = slice(g * ROWS, (g + 1) * ROWS)
                pay_t = pe.tile([ROWS, 4], f32, name=f"PAY{g}")
                for c, bt in enumerate((TCX10b, TCY10b, LTW5b, LTH5b)):
                    tp_ps = pp.tile([ROWS, ROWS], f32, name=f"tp{g}_{c}", tag="ps")
                    nc.tensor.transpose(out=tp_ps[:], in_=bt[:, gsl], identity=ident[:])
                    nc.scalar.copy(out=pay_t[:, c : c + 1], in_=tp_ps[:, 0:1])
                PAYg.append(pay_t)
            rid128 = pe.tile([ROWS, 1], f32, name="rid128")
            nc.vector.tensor_scalar(
                out=rid128[:], in0=ridf[:], scalar1=128.0, scalar2=None, op0=OP.add
            )

            # ---------------- Phase 1: main IoU loop ----------------
            runmax = pe.tile([ROWS, T], f32, name="runmax")
            runarg = pe.tile([ROWS, T], f32, name="runarg")
            nc.vector.memset(runmax[:], -1.0)
            nc.vector.memset(runarg[:], 0.0)
            bto8 = pe.tile([ROWS, NT, 8], f32, name="bto8")
            pay_all = pe.tile([ROWS, NT, 4], f32, name="pay_all")

            p1_loop = tc.For_i(0, phase1_reps, 1) if phase1_reps > 1 else None
            if p1_loop is not None:
                p1_loop.__enter__()
            for j in range(NT):
                jj = slice(j, j + 1)
                ax = wp.tile([ROWS, T], f32, name="ax", tag="ax")
                nc.vector.tensor_scalar(
                    out=ax[:], in0=TX2b[:], scalar1=px2[:, jj], scalar2=None, op0=OP.min
                )
                bx = wp.tile([ROWS, T], f32, name="bx", tag="bx")
                nc.vector.tensor_scalar(
                    out=bx[:], in0=TX1b[:], scalar1=px1[:, jj], scalar2=None, op0=OP.max
                )
                wx = wp.tile([ROWS, T], f32, name="wx", tag="wx")
                nc.gpsimd.tensor_tensor(out=wx[:], in0=ax[:], in1=bx[:], op=OP.subtract)
                wr = wp.tile([ROWS, T], f32, name="wr", tag="wr")
                nc.scalar.activation(out=wr[:], in_=wx[:], func=AF.Relu)

                ay = wp.tile([ROWS, T], f32, name="ay", tag="ay")
                nc.vector.tensor_scalar(
                    out=ay[:], in0=TY2b[:], scalar1=py2[:, jj], scalar2=None, op0=OP.min
                )
                by = wp.tile([ROWS, T], f32, name="by", tag="by")
                nc.vector.tensor_scalar(
                    out=by[:], in0=TY1b[:], scalar1=py1[:, jj], scalar2=None, op0=OP.max
                )
                hy = wp.tile([ROWS, T], f32, name="hy", tag="hy")
                nc.gpsimd.tensor_tensor(out=hy[:], in0=ay[:], in1=by[:], op=OP.subtract)
                hr = wp.tile([ROWS, T], f32, name="hr", tag="hr")
                nc.scalar.activation(out=hr[:], in_=hy[:], func=AF.Relu)

                inter = wp.tile([ROWS, T], f32, name="inter", tag="inter")
                nc.vector.tensor_tensor(out=inter[:], in0=wr[:], in1=hr[:], op=OP.mult)
                S = wp.tile([ROWS, T], f32, name="S", tag="S")
                nc.scalar.activation(
                    out=S[:], in_=TAb[:], func=AF.Identity, bias=areap[:, jj], scale=1.0
                )
                lS = wp.tile([ROWS, T], f32, name="lS", tag="lS")
                nc.scalar.activation(out=lS[:], in_=S[:], func=AF.Ln)
                rS = wp.tile([ROWS, T], f32, name="rS", tag="rS")
                nc.scalar.activation(out=rS[:], in_=lS[:], func=AF.Exp, scale=-1.0)
                f = wp.tile([ROWS, T], f32, name="f", tag="f")
                nc.vector.tensor_tensor(out=f[:], in0=inter[:], in1=rS[:], op=OP.mult)

                nc.vector.max(bto8[:, j, :], f[:])
                idx8 = wp.tile([ROWS, 8], u32, name="idx8", tag="idx8")
                nc.vector.max_index(idx8[:], bto8[:, j, :], f[:])
                btif = wp.tile([ROWS, 1], f32, name="btif", tag="btif")
                nc.vector.tensor_copy(out=btif[:], in_=idx8[:, 0:1])
                # gather payload of best truth via PE: one-hot^T chunks @ PAY_g
                btb_ps = pp.tile([ROWS, ROWS], f32, name="btb_ps", tag="btb")
                nc.tensor.transpose(
                    out=btb_ps[:],
                    in_=btif[:, 0:1].to_broadcast([ROWS, ROWS]),
                    identity=ident[:],
                )
                btb_sb = wp.tile([ROWS, ROWS], f32, name="btb_sb", tag="btb_sb")
                nc.scalar.copy(out=btb_sb[:], in_=btb_ps[:])
                pay_ps = pp.tile([ROWS, 4], f32, name="pay_ps", tag="pay")
                for g in range(2):
                    ohT = wp.tile([ROWS, ROWS], f32, name=f"ohT{g}", tag=f"ohT{g}")
                    nc.vector.tensor_scalar(
                        out=ohT[:],
                        in0=btb_sb[:],
                        scalar1=(ridf[:] if g == 0 else rid128[:]),
                        scalar2=None,
                        op0=OP.is_equal,
                    )
                    nc.tensor.matmul(
                        out=pay_ps[:],
                        lhsT=ohT[:],
                        rhs=PAYg[g][:],
                        start=(g == 0),
                        stop=(g == 1),
                    )
                nc.scalar.copy(out=pay_all[:, j, :], in_=pay_ps[:])

                mask = wp.tile([ROWS, T], mybir.dt.uint8, name="mask", tag="mask")
                nc.vector.tensor_tensor(out=mask[:], in0=f[:], in1=runmax[:], op=OP.is_gt)
                nc.vector.tensor_tensor(
                    out=runmax[:], in0=runmax[:], in1=f[:], op=OP.max
                )
                v = wp.tile([ROWS, T], f32, name="v", tag="v")
                nc.vector.tensor_scalar(
                    out=v[:], in0=IOTA_PRI[:], scalar1=float(j), scalar2=None, op0=OP.add
                )
                nc.vector.copy_predicated(out=runarg[:], mask=mask[:], data=v[:])

            if p1_loop is not None:
                p1_loop.__exit__(None, None, None)

            # ---------------- Phase 2: best-prior endgame ----------------
            lval = []
            larg = []
            for g in range(2):
                gsl = slice(g * ROWS, (g + 1) * ROWS)
                psT = pp.tile([ROWS, ROWS], f32, name=f"psT{g}", tag="ps")
                nc.tensor.transpose(out=psT[:], in_=runmax[:, gsl], identity=ident[:])
                rmT = pe.tile([ROWS, ROWS], f32, name=f"rmT{g}")
                nc.scalar.copy(out=rmT[:], in_=psT[:])
                psT2 = pp.tile([ROWS, ROWS], f32, name=f"psT2{g}", tag="ps")
                nc.tensor.transpose(out=psT2[:], in_=runarg[:, gsl], identity=ident[:])
                raT = pe.tile([ROWS, ROWS], f32, name=f"raT{g}")
                nc.scalar.copy(out=raT[:], in_=psT2[:])

                m8 = pe.tile([ROWS, 8], f32, name=f"m8_{g}")
                nc.vector.max(m8[:], rmT[:])
                i8 = pe.tile([ROWS, 8], u32, name=f"i8_{g}")
                nc.vector.max_index(i8[:], m8[:], rmT[:])
                rstarf = pe.tile([ROWS, 1], f32, name=f"rstarf{g}")
                nc.vector.tensor_copy(out=rstarf[:], in_=i8[:, 0:1])
                ohr = wp.tile([ROWS, ROWS], f32, name=f"ohr{g}", tag="ohr")
                nc.vector.tensor_scalar(
                    out=ohr[:],
                    in0=IOTA_TF[:, 0:ROWS],
                    scalar1=rstarf[:],
                    scalar2=None,
                    op0=OP.is_equal,
                )
                trash2 = wp.tile([ROWS, ROWS], f32, name=f"trash2{g}", tag="trash2")
                la = pe.tile([ROWS, 1], f32, name=f"larg{g}")
                nc.vector.tensor_tensor(out=trash2[:], in0=ohr[:], in1=raT[:], op=OP.mult)
                nc.vector.tensor_reduce(
                    out=la[:], in_=trash2[:], axis=mybir.AxisListType.X, op=OP.add
                )
                lval.append(m8)
                larg.append(la)

            # AllGather of (val[256], arg[256])
            ag_in = dp.tile([2, T], f32, name="ag_in")
            for g in range(2):
                nc.sync.dma_start(
                    out=ag_in[0, g * ROWS : (g + 1) * ROWS], in_=lval[g][:, 0:1]
                )
                nc.sync.dma_start(
                    out=ag_in[1, g * ROWS : (g + 1) * ROWS], in_=larg[g][:, 0:1]
                )
            ag_out = dp.tile(
                [NCORES, 2, T], f32, name="ag_out", addr_space="Shared"
            )
            nc.gpsimd.collective_compute(
                "AllGather",
                mybir.AluOpType.bypass,
                ins=[ag_in[:]],
                outs=[ag_out[:]],
                replica_groups=RG,
            )

            p_star = []
            keep = []
            for g in range(2):
                gsl = slice(g * ROWS, (g + 1) * ROWS)
                vals8 = pe.tile([ROWS, NCORES], f32, name=f"vals8_{g}")
                args8 = pe.tile([ROWS, NCORES], f32, name=f"args8_{g}")
                vals8t = pe.tile([ROWS, NCORES], f32, name=f"vals8t_{g}")
                args8t = pe.tile([ROWS, NCORES], f32, name=f"args8t_{g}")
                nc.sync.dma_start(
                    out=vals8t[:], in_=ag_out[:, 0, gsl].rearrange("k t -> t k")
                )
                nc.sync.dma_start(
                    out=args8t[:], in_=ag_out[:, 1, gsl].rearrange("k t -> t k")
                )
                nc.scalar.copy(out=vals8[:], in_=vals8t[:])
                nc.scalar.copy(out=args8[:], in_=args8t[:])
                garg = pe.tile([ROWS, NCORES], f32, name=f"garg{g}")
                nc.vector.tensor_tensor(
                    out=garg[:], in0=args8[:], in1=OFF8[:], op=OP.add
                )
                gval = pe.tile([ROWS, NCORES], f32, name=f"gval{g}")
                nc.vector.tensor_copy(out=gval[:], in_=vals8[:])
                for width in (4, 2, 1):
                    va = gval[:, 0:width]
                    vb = gval[:, width : 2 * width]
                    mk = wp.tile([ROWS, width], mybir.dt.uint8, name=f"mk{g}_{width}", tag="mk")
                    nc.vector.tensor_tensor(out=mk[:], in0=vb, in1=va, op=OP.is_gt)
                    nc.vector.tensor_tensor(out=va, in0=va, in1=vb, op=OP.max)
                    nc.vector.copy_predicated(
                        out=garg[:, 0:width], mask=mk[:], data=garg[:, width : 2 * width]
                    )
                p_star.append(garg)

            # broadcast p_star over partitions: PSb[q, t] = p_star[t]
            PSb = pe.tile([ROWS, T], f32, name="PSb")
            for g in range(2):
                psb_ps = pp.tile([ROWS, ROWS], f32, name=f"psb_ps{g}", tag="ps")
                nc.tensor.transpose(
                    out=psb_ps[:],
                    in_=p_star[g][:, 0:1].to_broadcast([ROWS, ROWS]),
                    identity=ident[:],
                )
                nc.scalar.copy(out=PSb[:, g * ROWS : (g + 1) * ROWS], in_=psb_ps[:])

            for g in range(2):
                eqm = wp.tile([ROWS, T], f32, name=f"eqm{g}", tag="eqm")
                nc.vector.tensor_scalar(
                    out=eqm[:],
                    in0=PSb[:],
                    scalar1=p_star[g][:, 0:1],
                    scalar2=None,
                    op0=OP.is_equal,
                )
                rid_g = pe.tile([ROWS, 1], f32, name=f"rid_g{g}")
                nc.vector.tensor_scalar(
                    out=rid_g[:],
                    in0=ridf[:],
                    scalar1=float(g * ROWS),
                    scalar2=None,
                    op0=OP.add,
                )
                trg = wp.tile([ROWS, T], f32, name=f"trg{g}", tag="trg")
                nc.vector.tensor_scalar(
                    out=trg[:],
                    in0=IOTA_TF[:],
                    scalar1=rid_g[:],
                    scalar2=None,
                    op0=OP.is_gt,
                )
                anyl = pe.tile([ROWS, 1], f32, name=f"anyl{g}")
                trash3 = wp.tile([ROWS, T], f32, name=f"trash3{g}", tag="trash3")
                nc.vector.tensor_tensor(out=trash3[:], in0=eqm[:], in1=trg[:], op=OP.mult)
                nc.vector.tensor_reduce(
                    out=anyl[:], in_=trash3[:], axis=mybir.AxisListType.X, op=OP.max
                )
                kp = pe.tile([ROWS, 1], f32, name=f"keep{g}")
                nc.vector.tensor_scalar(
                    out=kp[:],
                    in0=anyl[:],
                    scalar1=-1.0,
                    scalar2=1.0,
                    op0=OP.mult,
                    op1=OP.add,
                )
                keep.append(kp)

            # ---------------- Phase 3: per-prior epilogue ----------------
            btoF = bto8[:, :, 0]  # [ROWS, NT] stride-8 view
            xf = pe.tile([ROWS, NT], f32, name="xf")
            nc.vector.tensor_scalar(
                out=xf[:], in0=btoF, scalar1=TH_F, scalar2=None, op0=OP.is_gt
            )
            ptcx = pay_all[:, :, 0]
            ptcy = pay_all[:, :, 1]
            ptlw = pay_all[:, :, 2]
            ptlh = pay_all[:, :, 3]
            e_tiles = []
            for idx, (pay, cen, rp) in enumerate(
                ((ptcx, pcx10, rpw), (ptcy, pcy10, rph))
            ):
                e = pe.tile([ROWS, NT], f32, name=f"e{idx}")
                nc.vector.tensor_tensor(out=e[:], in0=pay, in1=cen[:], op=OP.subtract)
                nc.vector.tensor_tensor(out=e[:], in0=e[:], in1=rp[:], op=OP.mult)
                e_tiles.append(e)
            for idx, (pay, lp) in enumerate(((ptlw, lpw5), (ptlh, lph5))):
                e = pe.tile([ROWS, NT], f32, name=f"e{idx + 2}")
                nc.vector.tensor_tensor(out=e[:], in0=pay, in1=lp[:], op=OP.subtract)
                e_tiles.append(e)

            l1u = pe.tile([ROWS, NT], f32, name="l1u")
            nc.vector.memset(l1u[:], 0.0)
            for idx, e in enumerate(e_tiles):
                ae = wp.tile([ROWS, NT], f32, name=f"ae{idx}", tag="ae")
                nc.scalar.activation(out=ae[:], in_=e[:], func=AF.Abs)
                m = wp.tile([ROWS, NT], f32, name=f"m{idx}", tag="m")
                nc.vector.tensor_scalar(
                    out=m[:], in0=ae[:], scalar1=1.0, scalar2=None, op0=OP.min
                )
                pq = wp.tile([ROWS, NT], f32, name=f"pq{idx}", tag="pq")
                nc.vector.tensor_tensor(out=pq[:], in0=m[:], in1=ae[:], op=OP.mult)
                rr = wp.tile([ROWS, NT], f32, name=f"rr{idx}", tag="rr")
                nc.vector.tensor_scalar(
                    out=rr[:],
                    in0=ae[:],
                    scalar1=1.0,
                    scalar2=0.0,
                    op0=OP.subtract,
                    op1=OP.max,
                )
                nc.vector.tensor_tensor(out=pq[:], in0=pq[:], in1=rr[:], op=OP.add)
                nc.vector.tensor_tensor(out=l1u[:], in0=l1u[:], in1=pq[:], op=OP.add)

            term = pe.tile([ROWS, NT], f32, name="term")
            nc.vector.tensor_tensor(out=term[:], in0=s_sb[:], in1=xf[:], op=OP.mult)
            nc.vector.tensor_tensor(out=term[:], in0=term[:], in1=l1u[:], op=OP.mult)
            pack3 = pe.tile([ROWS, 3], f32, name="pack3")
            nc.vector.tensor_reduce(
                out=pack3[:, 0:1], in_=term[:], axis=mybir.AxisListType.X, op=OP.add
            )
            nc.vector.tensor_reduce(
                out=pack3[:, 1:2], in_=xf[:], axis=mybir.AxisListType.X, op=OP.add
            )
            nc.vector.tensor_reduce(
                out=pack3[:, 2:3], in_=s_sb[:], axis=mybir.AxisListType.X, op=OP.add
            )
            sums_ps = pp.tile([1, 3], f32, name="sums_ps", tag="ps")
            nc.tensor.matmul(
                out=sums_ps[:], lhsT=ones128[:], rhs=pack3[:], start=True, stop=True
            )
            sums_sb = pe.tile([1, 3], f32, name="sums_sb")
            nc.scalar.copy(out=sums_sb[:], in_=sums_ps[:])

            # ---------------- Phase 4: best-prior corrections ----------------
            dnum_g = []
            dden_g = []
            for g in range(2):
                gsl = slice(g * ROWS, (g + 1) * ROWS)
                od = pe.tile([ROWS, 1], f32, name=f"od{g}")
                nc.vector.tensor_scalar(
                    out=od[:],
                    in0=p_star[g][:, 0:1],
                    scalar1=POFFb[:],
                    scalar2=None,
                    op0=OP.subtract,
                )
                o1 = wp.tile([ROWS, 1], f32, name=f"o1{g}", tag="o1")
                nc.vector.tensor_scalar(
                    out=o1[:], in0=od[:], scalar1=0.0, scalar2=None, op0=OP.is_ge
                )
                o2 = wp.tile([ROWS, 1], f32, name=f"o2{g}", tag="o2")
                nc.vector.tensor_scalar(
                    out=o2[:], in0=od[:], scalar1=float(PL), scalar2=None, op0=OP.is_lt
                )
                ownk = pe.tile([ROWS, 1], f32, name=f"ownk{g}")
                nc.vector.tensor_tensor(out=ownk[:], in0=o1[:], in1=o2[:], op=OP.mult)
                nc.vector.tensor_tensor(
                    out=ownk[:], in0=ownk[:], in1=keep[g][:], op=OP.mult
                )
                phf = pe.tile([ROWS, 1], f32, name=f"phf{g}")
                nc.vector.tensor_scalar(
                    out=phf[:],
                    in0=od[:],
                    scalar1=0.0,
                    scalar2=float(PL - 1),
                    op0=OP.max,
                    op1=OP.min,
                )
                phu = pe.tile([ROWS, 1], u32, name=f"phu{g}")
                nc.vector.tensor_copy(out=phu[:], in_=phf[:])
                rhat_u = pe.tile([ROWS, 1], u32, name=f"rhatu{g}")
                nc.vector.tensor_scalar(
                    out=rhat_u[:],
                    in0=phu[:],
                    scalar1=7,
                    scalar2=None,
                    op0=OP.logical_shift_right,
                )
                jhat_u = pe.tile([ROWS, 1], u32, name=f"jhatu{g}")
                nc.vector.tensor_scalar(
                    out=jhat_u[:],
                    in0=phu[:],
                    scalar1=127,
                    scalar2=None,
                    op0=OP.bitwise_and,
                )
                rhat_f = pe.tile([ROWS, 1], f32, name=f"rhatf{g}")
                nc.vector.tensor_copy(out=rhat_f[:], in_=rhat_u[:])
                jhat_f = pe.tile([ROWS, 1], f32, name=f"jhatf{g}")
                nc.vector.tensor_copy(out=jhat_f[:], in_=jhat_u[:])

                # gather prior raw data from DRAM
                glocst = pe.tile([ROWS, 2], f32, name=f"glocst{g}")
                nc.gpsimd.indirect_dma_start(
                    out=glocst[:],
                    out_offset=None,
                    in_=locs_ext[:],
                    in_offset=bass.IndirectOffsetOnAxis(ap=phu[:, 0:1], axis=0),
                )
                glocs = pe.tile([ROWS, 2], f32, name=f"glocs{g}")
                nc.scalar.copy(out=glocs[:], in_=glocst[:])
                gparamst = pe.tile([ROWS, 3], f32, name=f"gparamst{g}")
                nc.gpsimd.indirect_dma_start(
                    out=gparamst[:],
                    out_offset=None,
                    in_=params_ext[:],
                    in_offset=bass.IndirectOffsetOnAxis(ap=phu[:, 0:1], axis=0),
                )
                gparams = pe.tile([ROWS, 3], f32, name=f"gparams{g}")
                nc.scalar.copy(out=gparams[:], in_=gparamst[:])

                # per-truth payload columns: reuse the PAY_g tables
                tcols = [PAYg[g][:, c : c + 1] for c in range(4)]

                gcx = glocs[:, 0:1]
                gcy = glocs[:, 1:2]
                gw = gparams[:, 0:1]
                gh = gparams[:, 1:2]
                rgw = pe.tile([ROWS, 1], f32, name=f"rgw{g}")
                nc.vector.reciprocal(out=rgw[:], in_=gw)
                rgh = pe.tile([ROWS, 1], f32, name=f"rgh{g}")
                nc.vector.reciprocal(out=rgh[:], in_=gh)
                lgw5 = pe.tile([ROWS, 1], f32, name=f"lgw5{g}")
                nc.scalar.activation(out=lgw5[:], in_=gw, func=AF.Ln)
                nc.vector.tensor_scalar(
                    out=lgw5[:], in0=lgw5[:], scalar1=5.0, scalar2=None, op0=OP.mult
                )
                lgh5 = pe.tile([ROWS, 1], f32, name=f"lgh5{g}")
                nc.scalar.activation(out=lgh5[:], in_=gh, func=AF.Ln)
                nc.vector.tensor_scalar(
                    out=lgh5[:], in0=lgh5[:], scalar1=5.0, scalar2=None, op0=OP.mult
                )

                encs = []
                for cc, (tcol, gcen, rg_) in enumerate(
                    ((tcols[0], gcx, rgw), (tcols[1], gcy, rgh))
                ):
                    gx10 = wp.tile([ROWS, 1], f32, name=f"gx10_{g}_{cc}", tag="gx10")
                    nc.vector.tensor_scalar(
                        out=gx10[:], in0=gcen, scalar1=10.0, scalar2=None, op0=OP.mult
                    )
                    en = pe.tile([ROWS, 1], f32, name=f"en{g}_{cc}")
                    nc.vector.tensor_tensor(
                        out=en[:], in0=tcol, in1=gx10[:], op=OP.subtract
                    )
                    nc.vector.tensor_tensor(out=en[:], in0=en[:], in1=rg_[:], op=OP.mult)
                    encs.append(en)
                for cc, (tcol, lg_) in enumerate(
                    ((tcols[2], lgw5), (tcols[3], lgh5))
                ):
                    en = pe.tile([ROWS, 1], f32, name=f"en{g}_{cc + 2}")
                    nc.vector.tensor_tensor(
                        out=en[:], in0=tcol, in1=lg_[:], op=OP.subtract
                    )
                    encs.append(en)

                l1n = pe.tile([ROWS, 1], f32, name=f"l1n{g}")
                nc.vector.memset(l1n[:], 0.0)
                for cc, en in enumerate(encs):
                    ae = wp.tile([ROWS, 1], f32, name=f"nae{g}_{cc}", tag="nae")
                    nc.scalar.activation(out=ae[:], in_=en[:], func=AF.Abs)
                    m = wp.tile([ROWS, 1], f32, name=f"nm{g}_{cc}", tag="nm")
                    nc.vector.tensor_scalar(
                        out=m[:], in0=ae[:], scalar1=1.0, scalar2=None, op0=OP.min
                    )
                    pq = wp.tile([ROWS, 1], f32, name=f"npq{g}_{cc}", tag="npq")
                    nc.vector.tensor_tensor(out=pq[:], in0=m[:], in1=ae[:], op=OP.mult)
                    rr = wp.tile([ROWS, 1], f32, name=f"nrr{g}_{cc}", tag="nrr")
                    nc.vector.tensor_scalar(
                        out=rr[:],
                        in0=ae[:],
                        scalar1=1.0,
                        scalar2=0.0,
                        op0=OP.subtract,
                        op1=OP.max,
                    )
                    nc.vector.tensor_tensor(out=pq[:], in0=pq[:], in1=rr[:], op=OP.add)
                    nc.vector.tensor_tensor(out=l1n[:], in0=l1n[:], in1=pq[:], op=OP.add)

                # sandwich gathers of l1u, xf, s at (rhat, jhat)
                rhb_ps = pp.tile([ROWS, ROWS], f32, name=f"rhb_ps{g}", tag="ps")
                nc.tensor.transpose(
                    out=rhb_ps[:],
                    in_=rhat_f[:, 0:1].to_broadcast([ROWS, ROWS]),
                    identity=ident[:],
                )
                rhb = pe.tile([ROWS, ROWS], f32, name=f"rhb{g}")
                nc.scalar.copy(out=rhb[:], in_=rhb_ps[:])
                E_row = pe.tile([ROWS, ROWS], f32, name=f"E_row{g}")
                nc.vector.tensor_scalar(
                    out=E_row[:],
                    in0=rhb[:],
                    scalar1=ridf[:],
                    scalar2=None,
                    op0=OP.is_equal,
                )
                E_colT = pe.tile([ROWS, ROWS], f32, name=f"E_colT{g}")
                nc.vector.tensor_scalar(
                    out=E_colT[:],
                    in0=IOTA_TF[:, 0:ROWS],
                    scalar1=jhat_f[:],
                    scalar2=None,
                    op0=OP.is_equal,
                )

                gath = []
                for ai, arr in enumerate((l1u, xf, s_sb)):
                    B_ps = pp.tile([ROWS, ROWS], f32, name=f"B_ps{g}_{ai}", tag="ps")
                    nc.tensor.matmul(
                        out=B_ps[:], lhsT=arr[:], rhs=E_row[:], start=True, stop=True
                    )
                    Bs = wp.tile([ROWS, ROWS], f32, name=f"Bs{g}_{ai}", tag="Bs")
                    nc.scalar.copy(out=Bs[:], in_=B_ps[:])
                    BT_ps = pp.tile(
                        [ROWS, ROWS], f32, name=f"BT_ps{g}_{ai}", tag="ps"
                    )
                    nc.tensor.transpose(out=BT_ps[:], in_=Bs[:], identity=ident[:])
                    BTs = wp.tile([ROWS, ROWS], f32, name=f"BTs{g}_{ai}", tag="BTs")
                    nc.scalar.copy(out=BTs[:], in_=BT_ps[:])
                    acc = pe.tile([ROWS, 1], f32, name=f"gath{g}_{ai}")
                    trash4 = wp.tile(
                        [ROWS, ROWS], f32, name=f"trash4{g}_{ai}", tag="trash4"
                    )
                    nc.vector.tensor_tensor(
                        out=trash4[:], in0=BTs[:], in1=E_colT[:], op=OP.mult
                    )
                    nc.vector.tensor_reduce(
                        out=acc[:], in_=trash4[:], axis=mybir.AxisListType.X, op=OP.add
                    )
                    gath.append(acc)
                l1_at, xf_at, s_at = gath

                w1 = wp.tile([ROWS, 1], f32, name=f"w1{g}", tag="w1")
                nc.vector.tensor_scalar(
                    out=w1[:], in0=l1n[:], scalar1=K_VAL, scalar2=None, op0=OP.mult
                )
                w2 = wp.tile([ROWS, 1], f32, name=f"w2{g}", tag="w2")
                nc.vector.tensor_tensor(out=w2[:], in0=xf_at[:], in1=l1_at[:], op=OP.mult)
                nc.vector.tensor_tensor(out=w1[:], in0=w1[:], in1=w2[:], op=OP.subtract)
                nc.vector.tensor_tensor(out=w1[:], in0=w1[:], in1=s_at[:], op=OP.mult)
                dn = pe.tile([ROWS, 1], f32, name=f"dn{g}")
                nc.vector.tensor_tensor(out=dn[:], in0=w1[:], in1=ownk[:], op=OP.mult)
                dd = pe.tile([ROWS, 1], f32, name=f"dd{g}")
                nc.vector.tensor_scalar(
                    out=dd[:],
                    in0=xf_at[:],
                    scalar1=-1.0,
                    scalar2=K_VAL,
                    op0=OP.mult,
                    op1=OP.add,
                )
                nc.vector.tensor_tensor(out=dd[:], in0=dd[:], in1=ownk[:], op=OP.mult)
                dnum_g.append(dn)
                dden_g.append(dd)

            pack2 = pe.tile([ROWS, 2], f32, name="pack2")
            nc.vector.tensor_tensor(
                out=pack2[:, 0:1], in0=dnum_g[0][:], in1=dnum_g[1][:], op=OP.add
            )
            nc.vector.tensor_tensor(
                out=pack2[:, 1:2], in0=dden_g[0][:], in1=dden_g[1][:], op=OP.add
            )
            sums2_ps = pp.tile([1, 2], f32, name="sums2_ps", tag="ps")
            nc.tensor.matmul(
                out=sums2_ps[:], lhsT=ones128[:], rhs=pack2[:], start=True, stop=True
            )
            sums2_sb = pe.tile([1, 2], f32, name="sums2_sb")
            nc.scalar.copy(out=sums2_sb[:], in_=sums2_ps[:])

            # ---------------- Phase 5: final AllReduce + loss ----------------
            asm = pe.tile([1, 8], f32, name="asm")
            nc.vector.memset(asm[:], 0.0)
            nc.vector.tensor_copy(out=asm[:, 0:3], in_=sums_sb[:])
            nc.vector.tensor_copy(out=asm[:, 3:5], in_=sums2_sb[:])
            ar_in = dp.tile([1, 8], f32, name="ar_in")
            nc.sync.dma_start(out=ar_in[:], in_=asm[:])
            ar_out = dp.tile([1, 8], f32, name="ar_out", addr_space="Shared")
            nc.gpsimd.collective_compute(
                "AllReduce",
                mybir.AluOpType.add,
                ins=[ar_in[:]],
                outs=[ar_out[:]],
                replica_groups=RG,
            )
            gsumt = pe.tile([1, 8], f32, name="gsumt")
            nc.sync.dma_start(out=gsumt[:], in_=ar_out[:])
            gsum = pe.tile([1, 8], f32, name="gsum")
            nc.scalar.copy(out=gsum[:], in_=gsumt[:])

            num = pe.tile([1, 1], f32, name="num")
            nc.vector.tensor_tensor(
                out=num[:], in0=gsum[:, 0:1], in1=gsum[:, 3:4], op=OP.add
            )
            nc.vector.tensor_scalar(
                out=num[:], in0=num[:], scalar1=0.5, scalar2=None, op0=OP.mult
            )
            nc.vector.tensor_tensor(
                out=num[:], in0=num[:], in1=gsum[:, 2:3], op=OP.add
            )  # + BETA * sum(sigmoid)
            den = pe.tile([1, 1], f32, name="den")
            nc.vector.tensor_tensor(
                out=den[:], in0=gsum[:, 1:2], in1=gsum[:, 4:5], op=OP.add
            )
            rden = pe.tile([1, 1], f32, name="rden")
            nc.vector.reciprocal(out=rden[:], in_=den[:])
            loss = pe.tile([1, 1], f32, name="loss")
            nc.vector.tensor_tensor(out=loss[:], in0=num[:], in1=rden[:], op=OP.mult)
            nc.sync.dma_start(out=out_ext[:], in_=loss[:])

    _split_waits(nc)
    return nc


def _split_waits(nc):
    """This toolchain's codegen accepts only one embedded sem-wait per
    instruction; hoist extra waits into standalone EventSemaphore
    instructions on the same engine (same blocking semantics)."""
    import orjson

    import copy as _copy

    d = orjson.loads(nc.to_json_bytes())
    ctr = 0
    for fn in d.get("functions", []):
        for bb in fn.get("blocks", []):
            out = []
            for ins in bb.get("instructions", []):
                if (
                    ins.get("opcode") == "ISA"
                    and ins.get("op_name") == "EVENT_SEMAPHORE_RANGE_CLEAR"
                ):
                    # codegen rejects clear ranges wider than 16 sems; split.
                    first, last = ins["instr"][13], ins["instr"][14]
                    if last - first + 1 > 16:
                        lo = first
                        while lo <= last:
                            hi = min(lo + 15, last)
                            ctr += 1
                            part = _copy.deepcopy(ins)
                            part["name"] = f"{ins['name']}_rc{ctr}"
                            part["instr"] = list(ins["instr"])
                            part["instr"][13] = lo
                            part["instr"][14] = hi
                            if lo != first:
                                part["sync_info"] = {"on_wait": [], "on_update": []}
                            out.append(part)
                            lo = hi + 1
                        continue
                si = ins.get("sync_info")
                ow = (si or {}).get("on_wait") or []
                if si and len(ow) > 1 and "engine" in ins:
                    for w in ow[:-1]:
                        ctr += 1
                        ev = {
                            "engine": ins["engine"],
                            "ins": [],
                            "outs": [],
                            "name": f"antsplit_{ctr}",
                            "opcode": "EventSemaphore",
                            "sync_info": {"on_wait": [w], "on_update": []},
                        }
                        if "debug" in ins:
                            ev["debug"] = ins["debug"]
                        out.append(ev)
                    si["on_wait"] = [ow[-1]]
                out.append(ins)
            bb["instructions"] = out
    blob = orjson.dumps(d)
    nc.to_json_bytes = lambda: blob
    return nc


def kernel(**inputs):
    locs = np.ascontiguousarray(np.asarray(inputs["locs"], dtype=np.float32))
    params = np.ascontiguousarray(np.asarray(inputs["params"], dtype=np.float32))
    truths = np.ascontiguousarray(np.asarray(inputs["truths"], dtype=np.float32))

    if "nc" not in _CACHE:
        _CACHE["nc"] = _build()
    nc = _CACHE["nc"]

    in_maps = []
    for c in range(NCORES):
        in_maps.append(
            {
                "locs": locs[c * PL : (c + 1) * PL],
                "params": params[c * PL : (c + 1) * PL],
                "truths": truths,
                "poff": np.array([[c * PL]], dtype=np.float32),
            }
        )

    from concourse.bass_utils import run_bass_kernel_spmd

    res = run_bass_kernel_spmd(nc, in_maps, core_ids=list(range(NCORES)))
    out = np.asarray(res.results[0]["out"], dtype=np.float32)
    return out.reshape(())


if __name__ == "__main__":
    sys.path.insert(0, "/root/problem")
    import reference

    inputs = {k: np.asarray(v) for k, v in reference.setup_inputs().items()}
    expected = np.asarray(reference.reference(**inputs))
    actual = kernel(**inputs)
    rel = abs(float(actual) - float(expected)) / max(abs(float(expected)), 1e-12)
    print("expected:", expected, "actual:", actual, "rel_err:", rel)



# revision 17
# speedup vs baseline: 1.2635x; 1.2635x over previous
import sys

sys.path.insert(0, "/opt/trn_rl_repo")

import numpy as np

# Problem constants (hardcoded per spec nn_AdaptivePriorBoxesLoss)
P_TOT = 131072
T = 256
NCORES = 8
PL = P_TOT // NCORES  # 16384 priors per core
ROWS = 128
NT = PL // ROWS  # 128 tiles per core; local prior p = r*NT + j
TH_F = 2.0 / 7.0  # iou > 0.4  <=>  inter/(areaA+areaB) > 2/7
LTH_F = float(np.log(2.0 / 7.0))  # same threshold in log domain
K_VAL = 2.5
BETA = 1.0
MASK_ON_GP = True  # best-prior mask (is_gt) on gpsimd; flip to False if wrong

_CACHE = {}


def _build():
    from concourse import bass, mybir, tile
    from concourse.masks import make_identity

    f32 = mybir.dt.float32
    AF = mybir.ActivationFunctionType
    OP = mybir.AluOpType

    nc = bass.Bass()
    locs_ext = nc.declare_dram_parameter("locs", [PL, 2], f32, isOutput=False)
    params_ext = nc.declare_dram_parameter("params", [PL, 3], f32, isOutput=False)
    truths_ext = nc.declare_dram_parameter("truths", [T, 4], f32, isOutput=False)
    poff_ext = nc.declare_dram_parameter("poff", [1, 1], f32, isOutput=False)
    out_ext = nc.declare_dram_parameter("out", [1, 1], f32, isOutput=True)

    RG = [list(range(NCORES))]

    with tile.TileContext(nc) as tc:
        with (
            tc.tile_pool(name="persist", bufs=1) as pe,
            tc.tile_pool(name="work", bufs=3) as wp,
            tc.tile_pool(name="psum", bufs=2, space="PSUM") as pp,
            tc.tile_pool(name="dram", bufs=1, space="DRAM") as dp,
        ):
            # ---------------- Phase 0: prep ----------------
            locs_sb = pe.tile([ROWS, NT, 2], f32, name="locs_sb")
            params_sb = pe.tile([ROWS, NT, 3], f32, name="params_sb")
            nc.sync.dma_start(
                out=locs_sb[:], in_=locs_ext[:].rearrange("(r j) c -> r j c", r=ROWS)
            )
            nc.sync.dma_start(
                out=params_sb[:],
                in_=params_ext[:].rearrange("(r j) c -> r j c", r=ROWS),
            )

            ones128 = pe.tile([ROWS, 1], f32, name="ones128")
            nc.vector.memset(ones128[:], 1.0)
            ident = pe.tile([ROWS, ROWS], f32, name="ident")
            make_identity(nc, ident[:])

            # truths land contiguously in partition 0; PE ones-matmul
            # broadcasts each column across all partitions (the DMA
            # partition-broadcast path costs ~33us per tensor).
            tr_row = pe.tile([1, T * 4], f32, name="tr_row")
            nc.sync.dma_start(
                out=tr_row[:], in_=truths_ext[:].rearrange("t c -> (t c)")[None, :]
            )
            poff_row = pe.tile([1, 1], f32, name="poff_row")
            nc.sync.dma_start(out=poff_row[:], in_=poff_ext[:])
            ones_row = pe.tile([1, ROWS], f32, name="ones_row")
            nc.vector.memset(ones_row[:], 1.0)

            def bcast_row(src_ap, name, n=T):
                ps = pp.tile([ROWS, n], f32, name=f"bc_{name}", tag="bc")
                nc.tensor.matmul(
                    out=ps[:], lhsT=ones_row[:], rhs=src_ap, start=True, stop=True
                )
                sb = pe.tile([ROWS, n], f32, name=name)
                nc.scalar.copy(out=sb[:], in_=ps[:])
                return sb

            TX1b = bcast_row(tr_row[0:1, 0 : T * 4 : 4], "TX1b")
            TY1b = bcast_row(tr_row[0:1, 1 : T * 4 : 4], "TY1b")
            TX2b = bcast_row(tr_row[0:1, 2 : T * 4 : 4], "TX2b")
            TY2b = bcast_row(tr_row[0:1, 3 : T * 4 : 4], "TY2b")
            POFFb = bcast_row(poff_row[0:1, 0:1], "POFFb", n=1)

            TWb = pe.tile([ROWS, T], f32, name="TWb")
            THb = pe.tile([ROWS, T], f32, name="THb")
            TAb = pe.tile([ROWS, T], f32, name="TAb")
            TCX10b = pe.tile([ROWS, T], f32, name="TCX10b")
            TCY10b = pe.tile([ROWS, T], f32, name="TCY10b")
            LTW5b = pe.tile([ROWS, T], f32, name="LTW5b")
            LTH5b = pe.tile([ROWS, T], f32, name="LTH5b")
            nc.vector.tensor_tensor(out=TWb[:], in0=TX2b[:], in1=TX1b[:], op=OP.subtract)
            nc.vector.tensor_tensor(out=THb[:], in0=TY2b[:], in1=TY1b[:], op=OP.subtract)
            nc.vector.tensor_tensor(out=TAb[:], in0=TWb[:], in1=THb[:], op=OP.mult)
            nc.vector.tensor_tensor(out=TCX10b[:], in0=TX1b[:], in1=TX2b[:], op=OP.add)
            nc.vector.tensor_scalar(
                out=TCX10b[:], in0=TCX10b[:], scalar1=5.0, scalar2=None, op0=OP.mult
            )
            nc.vector.tensor_tensor(out=TCY10b[:], in0=TY1b[:], in1=TY2b[:], op=OP.add)
            nc.vector.tensor_scalar(
                out=TCY10b[:], in0=TCY10b[:], scalar1=5.0, scalar2=None, op0=OP.mult
            )
            nc.scalar.activation(out=LTW5b[:], in_=TWb[:], func=AF.Ln)
            nc.vector.tensor_scalar(
                out=LTW5b[:], in0=LTW5b[:], scalar1=5.0, scalar2=None, op0=OP.mult
            )
            nc.scalar.activation(out=LTH5b[:], in_=THb[:], func=AF.Ln)
            nc.vector.tensor_scalar(
                out=LTH5b[:], in0=LTH5b[:], scalar1=5.0, scalar2=None, op0=OP.mult
            )

            # iotas
            it_i = pe.tile([ROWS, T], mybir.dt.int32, name="it_i")
            nc.gpsimd.iota(it_i[:], [[1, T]], base=0, channel_multiplier=0)
            IOTA_TF = pe.tile([ROWS, T], f32, name="IOTA_TF")
            nc.vector.tensor_copy(out=IOTA_TF[:], in_=it_i[:])
            rid_i = pe.tile([ROWS, 1], mybir.dt.int32, name="rid_i")
            nc.gpsimd.iota(rid_i[:], [[0, 1]], base=0, channel_multiplier=1)
            ridf = pe.tile([ROWS, 1], f32, name="ridf")
            nc.vector.tensor_copy(out=ridf[:], in_=rid_i[:])
            off8_i = pe.tile([ROWS, 8], mybir.dt.int32, name="off8_i")
            nc.gpsimd.iota(off8_i[:], [[PL, 8]], base=0, channel_multiplier=0)
            OFF8 = pe.tile([ROWS, 8], f32, name="OFF8")
            nc.vector.tensor_copy(out=OFF8[:], in_=off8_i[:])
            # T2[q, g] = q + 128*g : compare target for transposed one-hot
            T2 = pe.tile([ROWS, 2], f32, name="T2")
            nc.vector.tensor_copy(out=T2[:, 0:1], in_=ridf[:])
            nc.vector.tensor_scalar(
                out=T2[:, 1:2], in0=ridf[:], scalar1=128.0, scalar2=None, op0=OP.add
            )
            rid128 = pe.tile([ROWS, 1], f32, name="rid128")
            nc.vector.tensor_scalar(
                out=rid128[:], in0=ridf[:], scalar1=128.0, scalar2=None, op0=OP.add
            )

            # per-prior derived arrays [ROWS, NT]
            locs_l = pe.tile([ROWS, NT, 2], f32, name="locs_l")
            nc.scalar.copy(out=locs_l[:], in_=locs_sb[:])
            params_l = pe.tile([ROWS, NT, 3], f32, name="params_l")
            nc.scalar.copy(out=params_l[:], in_=params_sb[:])
            cxv = locs_l[:, :, 0]
            cyv = locs_l[:, :, 1]
            wv = params_l[:, :, 0]
            hv = params_l[:, :, 1]
            av = params_l[:, :, 2]

            halfw = pe.tile([ROWS, NT], f32, name="halfw")
            halfh = pe.tile([ROWS, NT], f32, name="halfh")
            px1 = pe.tile([ROWS, NT], f32, name="px1")
            px2 = pe.tile([ROWS, NT], f32, name="px2")
            py1 = pe.tile([ROWS, NT], f32, name="py1")
            py2 = pe.tile([ROWS, NT], f32, name="py2")
            areap = pe.tile([ROWS, NT], f32, name="areap")
            pcx10 = pe.tile([ROWS, NT], f32, name="pcx10")
            pcy10 = pe.tile([ROWS, NT], f32, name="pcy10")
            rpw = pe.tile([ROWS, NT], f32, name="rpw")
            rph = pe.tile([ROWS, NT], f32, name="rph")
            lpw5 = pe.tile([ROWS, NT], f32, name="lpw5")
            lph5 = pe.tile([ROWS, NT], f32, name="lph5")
            s_sb = pe.tile([ROWS, NT], f32, name="s_sb")

            nc.vector.tensor_scalar(
                out=halfw[:], in0=wv, scalar1=0.5, scalar2=None, op0=OP.mult
            )
            nc.vector.tensor_scalar(
                out=halfh[:], in0=hv, scalar1=0.5, scalar2=None, op0=OP.mult
            )
            nc.vector.tensor_tensor(out=px1[:], in0=cxv, in1=halfw[:], op=OP.subtract)
            nc.vector.tensor_tensor(out=px2[:], in0=cxv, in1=halfw[:], op=OP.add)
            nc.vector.tensor_tensor(out=py1[:], in0=cyv, in1=halfh[:], op=OP.subtract)
            nc.vector.tensor_tensor(out=py2[:], in0=cyv, in1=halfh[:], op=OP.add)
            nc.vector.tensor_tensor(out=areap[:], in0=wv, in1=hv, op=OP.mult)
            nc.vector.tensor_scalar(
                out=pcx10[:], in0=cxv, scalar1=10.0, scalar2=None, op0=OP.mult
            )
            nc.vector.tensor_scalar(
                out=pcy10[:], in0=cyv, scalar1=10.0, scalar2=None, op0=OP.mult
            )
            nc.vector.reciprocal(out=rpw[:], in_=wv)
            nc.vector.reciprocal(out=rph[:], in_=hv)
            nc.scalar.activation(out=lpw5[:], in_=wv, func=AF.Ln)
            nc.vector.tensor_scalar(
                out=lpw5[:], in0=lpw5[:], scalar1=5.0, scalar2=None, op0=OP.mult
            )
            nc.scalar.activation(out=lph5[:], in_=hv, func=AF.Ln)
            nc.vector.tensor_scalar(
                out=lph5[:], in0=lph5[:], scalar1=5.0, scalar2=None, op0=OP.mult
            )
            nc.scalar.activation(out=s_sb[:], in_=av, func=AF.Sigmoid)

            # payload tables PAY_g[t, c] for c in (tcx10, tcy10, ltw5, lth5)
            PAYg = []
            for g in range(2):
                gsl = slice(g * ROWS, (g + 1) * ROWS)
                pay_t = pe.tile([ROWS, 4], f32, name=f"PAY{g}")
                for c, bt in enumerate((TCX10b, TCY10b, LTW5b, LTH5b)):
                    tp_ps = pp.tile([ROWS, ROWS], f32, name=f"tp{g}_{c}", tag="ps")
                    nc.tensor.transpose(out=tp_ps[:], in_=bt[:, gsl], identity=ident[:])
                    nc.scalar.copy(out=pay_t[:, c : c + 1], in_=tp_ps[:, 0:1])
                PAYg.append(pay_t)

            # ---------------- Phase 1: main IoU loop ----------------
            runmax = pe.tile([ROWS, T], f32, name="runmax")
            runarg = pe.tile([ROWS, T], f32, name="runarg")
            nc.vector.memset(runmax[:], -1.0)
            nc.vector.memset(runarg[:], 0.0)
            m8_all = pe.tile([ROWS, NT, 8], f32, name="m8_all")
            pay_all = pe.tile([ROWS, NT, 4], f32, name="pay_all")

            for j in range(NT):
                jj = slice(j, j + 1)
                # x-extent: wx = min(TX2,px2) - max(TX1,px1)  (relu deferred)
                bx = wp.tile([ROWS, T], f32, name="bx", tag="bx")
                nc.vector.tensor_scalar(
                    out=bx[:], in0=TX1b[:], scalar1=px1[:, jj], scalar2=None, op0=OP.max
                )
                wx = wp.tile([ROWS, T], f32, name="wx", tag="wx")
                nc.vector.scalar_tensor_tensor(
                    out=wx[:],
                    in0=TX2b[:],
                    scalar=px2[:, jj],
                    in1=bx[:],
                    op0=OP.min,
                    op1=OP.subtract,
                )
                # y-extent, relu on scalar engine
                by = wp.tile([ROWS, T], f32, name="by", tag="by")
                nc.vector.tensor_scalar(
                    out=by[:], in0=TY1b[:], scalar1=py1[:, jj], scalar2=None, op0=OP.max
                )
                hy = wp.tile([ROWS, T], f32, name="hy", tag="hy")
                nc.vector.scalar_tensor_tensor(
                    out=hy[:],
                    in0=TY2b[:],
                    scalar=py2[:, jj],
                    in1=by[:],
                    op0=OP.min,
                    op1=OP.subtract,
                )
                hr = wp.tile([ROWS, T], f32, name="hr", tag="hr")
                nc.scalar.activation(out=hr[:], in_=hy[:], func=AF.Relu)
                # inter = relu(wx) * hr
                inter = wp.tile([ROWS, T], f32, name="inter", tag="inter")
                nc.vector.scalar_tensor_tensor(
                    out=inter[:],
                    in0=wx[:],
                    scalar=0.0,
                    in1=hr[:],
                    op0=OP.max,
                    op1=OP.mult,
                )
                # log-domain IoU score: f = ln(inter) - ln(areaT + areaP)
                # (monotone in inter/A, so max/argmax/threshold carry over)
                lnA = wp.tile([ROWS, T], f32, name="lnA", tag="lnA")
                nc.scalar.activation(
                    out=lnA[:], in_=TAb[:], func=AF.Ln, bias=areap[:, jj], scale=1.0
                )
                lnI = wp.tile([ROWS, T], f32, name="lnI", tag="lnI")
                nc.scalar.activation(out=lnI[:], in_=inter[:], func=AF.Ln)
                f = wp.tile([ROWS, T], f32, name="f", tag="f")
                nc.gpsimd.tensor_tensor(
                    out=f[:], in0=lnI[:], in1=lnA[:], op=OP.subtract
                )
                nc.vector.max(m8_all[:, j, :], f[:])
                # best-truth index = sum_t (f==m)*t
                btif = wp.tile([ROWS, 1], f32, name="btif", tag="btif")
                trash = wp.tile([ROWS, T], f32, name="trash", tag="trash")
                nc.vector.scalar_tensor_tensor(
                    out=trash[:],
                    in0=f[:],
                    scalar=m8_all[:, j, 0:1],
                    in1=IOTA_TF[:],
                    op0=OP.is_equal,
                    op1=OP.mult,
                    accum_out=btif[:],
                )
                # gather payload of best truth via PE: one-hot^T chunks @ PAY_g
                btb_ps = pp.tile([ROWS, ROWS], f32, name="btb_ps", tag="btb")
                nc.tensor.transpose(
                    out=btb_ps[:],
                    in_=btif[:, 0:1].to_broadcast([ROWS, ROWS]),
                    identity=ident[:],
                )
                ohT2 = wp.tile([ROWS, 2, ROWS], f32, name="ohT2", tag="ohT2")
                nc.vector.tensor_tensor(
                    out=ohT2[:],
                    in0=btb_ps[:, None, :].to_broadcast([ROWS, 2, ROWS]),
                    in1=T2[:, :, None].to_broadcast([ROWS, 2, ROWS]),
                    op=OP.is_equal,
                )
                pay_ps = pp.tile([ROWS, 4], f32, name="pay_ps", tag="pay")
                for g in range(2):
                    nc.tensor.matmul(
                        out=pay_ps[:],
                        lhsT=ohT2[:, g, :],
                        rhs=PAYg[g][:],
                        start=(g == 0),
                        stop=(g == 1),
                    )
                nc.scalar.copy(out=pay_all[:, j, :], in_=pay_ps[:])

                # best-prior running argmax over local j (runarg stores j).
                # Pool rejects comparison ALU ops, so the mask stays on
                # vector (u8 for copy_predicated).
                mask = wp.tile([ROWS, T], mybir.dt.uint8, name="mask", tag="mask")
                nc.vector.tensor_tensor(
                    out=mask[:], in0=f[:], in1=runmax[:], op=OP.is_gt
                )
                nc.vector.tensor_tensor(
                    out=runmax[:], in0=runmax[:], in1=f[:], op=OP.max
                )
                nc.vector.copy_predicated(
                    out=runarg[:],
                    mask=mask[:],
                    data=IOTA_TF[:, jj].to_broadcast([ROWS, T]),
                )

            # ---------------- Phase 2: best-prior endgame ----------------
            lval = []
            larg = []
            for g in range(2):
                gsl = slice(g * ROWS, (g + 1) * ROWS)
                psT = pp.tile([ROWS, ROWS], f32, name=f"psT{g}", tag="ps")
                nc.tensor.transpose(out=psT[:], in_=runmax[:, gsl], identity=ident[:])
                rmT = pe.tile([ROWS, ROWS], f32, name=f"rmT{g}")
                nc.scalar.copy(out=rmT[:], in_=psT[:])
                psT2 = pp.tile([ROWS, ROWS], f32, name=f"psT2{g}", tag="ps")
                nc.tensor.transpose(out=psT2[:], in_=runarg[:, gsl], identity=ident[:])
                raT = pe.tile([ROWS, ROWS], f32, name=f"raT{g}")
                nc.scalar.copy(out=raT[:], in_=psT2[:])

                m8 = pe.tile([ROWS, 8], f32, name=f"m8_{g}")
                nc.vector.max(m8[:], rmT[:])
                i8 = pe.tile([ROWS, 8], mybir.dt.uint32, name=f"i8_{g}")
                nc.vector.max_index(i8[:], m8[:], rmT[:])
                rstarf = pe.tile([ROWS, 1], f32, name=f"rstarf{g}")
                nc.vector.tensor_copy(out=rstarf[:], in_=i8[:, 0:1])
                ohr = wp.tile([ROWS, ROWS], f32, name=f"ohr{g}", tag="ohr")
                nc.vector.tensor_scalar(
                    out=ohr[:],
                    in0=IOTA_TF[:, 0:ROWS],
                    scalar1=rstarf[:],
                    scalar2=None,
                    op0=OP.is_equal,
                )
                trash2 = wp.tile([ROWS, ROWS], f32, name=f"trash2{g}", tag="trash2")
                laj = pe.tile([ROWS, 1], f32, name=f"laj{g}")
                nc.vector.tensor_tensor(out=trash2[:], in0=ohr[:], in1=raT[:], op=OP.mult)
                nc.vector.tensor_reduce(
                    out=laj[:], in_=trash2[:], axis=mybir.AxisListType.X, op=OP.add
                )
                # local p = rstar*128 + j
                la = pe.tile([ROWS, 1], f32, name=f"larg{g}")
                nc.vector.scalar_tensor_tensor(
                    out=la[:],
                    in0=rstarf[:],
                    scalar=128.0,
                    in1=laj[:],
                    op0=OP.mult,
                    op1=OP.add,
                )
                lval.append(m8)
                larg.append(la)

            # AllGather of (val[256], arg[256])
            ag_in = dp.tile([2, T], f32, name="ag_in")
            for g in range(2):
                nc.sync.dma_start(
                    out=ag_in[0, g * ROWS : (g + 1) * ROWS], in_=lval[g][:, 0:1]
                )
                nc.sync.dma_start(
                    out=ag_in[1, g * ROWS : (g + 1) * ROWS], in_=larg[g][:, 0:1]
                )
            ag_out = dp.tile(
                [NCORES, 2, T], f32, name="ag_out", addr_space="Shared"
            )
            nc.gpsimd.collective_compute(
                "AllGather",
                mybir.AluOpType.bypass,
                ins=[ag_in[:]],
                outs=[ag_out[:]],
                replica_groups=RG,
            )

            p_star = []
            keep = []
            for g in range(2):
                gsl = slice(g * ROWS, (g + 1) * ROWS)
                vals8 = pe.tile([ROWS, NCORES], f32, name=f"vals8_{g}")
                args8 = pe.tile([ROWS, NCORES], f32, name=f"args8_{g}")
                vals8t = pe.tile([ROWS, NCORES], f32, name=f"vals8t_{g}")
                args8t = pe.tile([ROWS, NCORES], f32, name=f"args8t_{g}")
                nc.sync.dma_start(
                    out=vals8t[:], in_=ag_out[:, 0, gsl].rearrange("k t -> t k")
                )
                nc.sync.dma_start(
                    out=args8t[:], in_=ag_out[:, 1, gsl].rearrange("k t -> t k")
                )
                nc.scalar.copy(out=vals8[:], in_=vals8t[:])
                nc.scalar.copy(out=args8[:], in_=args8t[:])
                garg = pe.tile([ROWS, NCORES], f32, name=f"garg{g}")
                nc.vector.tensor_tensor(
                    out=garg[:], in0=args8[:], in1=OFF8[:], op=OP.add
                )
                gval = pe.tile([ROWS, NCORES], f32, name=f"gval{g}")
                nc.vector.tensor_copy(out=gval[:], in_=vals8[:])
                for width in (4, 2, 1):
                    va = gval[:, 0:width]
                    vb = gval[:, width : 2 * width]
                    mk = wp.tile(
                        [ROWS, width], mybir.dt.uint8, name=f"mk{g}_{width}", tag="mk"
                    )
                    nc.vector.tensor_tensor(out=mk[:], in0=vb, in1=va, op=OP.is_gt)
                    nc.vector.tensor_tensor(out=va, in0=va, in1=vb, op=OP.max)
                    nc.vector.copy_predicated(
                        out=garg[:, 0:width], mask=mk[:], data=garg[:, width : 2 * width]
                    )
                p_star.append(garg)

            # broadcast p_star over partitions: PSb[q, t] = p_star[t]
            PSb = pe.tile([ROWS, T], f32, name="PSb")
            for g in range(2):
                psb_ps = pp.tile([ROWS, ROWS], f32, name=f"psb_ps{g}", tag="ps")
                nc.tensor.transpose(
                    out=psb_ps[:],
                    in_=p_star[g][:, 0:1].to_broadcast([ROWS, ROWS]),
                    identity=ident[:],
                )
                nc.scalar.copy(out=PSb[:, g * ROWS : (g + 1) * ROWS], in_=psb_ps[:])

            for g in range(2):
                eqm = wp.tile([ROWS, T], f32, name=f"eqm{g}", tag="eqm")
                nc.vector.tensor_scalar(
                    out=eqm[:],
                    in0=PSb[:],
                    scalar1=p_star[g][:, 0:1],
                    scalar2=None,
                    op0=OP.is_equal,
                )
                rid_g = pe.tile([ROWS, 1], f32, name=f"rid_g{g}")
                nc.vector.tensor_scalar(
                    out=rid_g[:],
                    in0=ridf[:],
                    scalar1=float(g * ROWS),
                    scalar2=None,
                    op0=OP.add,
                )
                trg = wp.tile([ROWS, T], f32, name=f"trg{g}", tag="trg")
                nc.vector.tensor_scalar(
                    out=trg[:],
                    in0=IOTA_TF[:],
                    scalar1=rid_g[:],
                    scalar2=None,
                    op0=OP.is_gt,
                )
                anyl = pe.tile([ROWS, 1], f32, name=f"anyl{g}")
                trash3 = wp.tile([ROWS, T], f32, name=f"trash3{g}", tag="trash3")
                nc.vector.tensor_tensor(out=trash3[:], in0=eqm[:], in1=trg[:], op=OP.mult)
                nc.vector.tensor_reduce(
                    out=anyl[:], in_=trash3[:], axis=mybir.AxisListType.X, op=OP.max
                )
                kp = pe.tile([ROWS, 1], f32, name=f"keep{g}")
                nc.vector.tensor_scalar(
                    out=kp[:],
                    in0=anyl[:],
                    scalar1=-1.0,
                    scalar2=1.0,
                    op0=OP.mult,
                    op1=OP.add,
                )
                keep.append(kp)

            # ---------------- Phase 3: per-prior epilogue ----------------
            xf = pe.tile([ROWS, NT], f32, name="xf")
            # m8_all col0 is ln(inter/A) max; threshold in log domain
            nc.vector.tensor_scalar(
                out=xf[:], in0=m8_all[:, :, 0], scalar1=LTH_F, scalar2=None, op0=OP.is_gt
            )
            ptcx = pay_all[:, :, 0]
            ptcy = pay_all[:, :, 1]
            ptlw = pay_all[:, :, 2]
            ptlh = pay_all[:, :, 3]
            e_tiles = []
            for idx, (pay, cen, rp) in enumerate(
                ((ptcx, pcx10, rpw), (ptcy, pcy10, rph))
            ):
                e = pe.tile([ROWS, NT], f32, name=f"e{idx}")
                nc.vector.tensor_tensor(out=e[:], in0=pay, in1=cen[:], op=OP.subtract)
                nc.vector.tensor_tensor(out=e[:], in0=e[:], in1=rp[:], op=OP.mult)
                e_tiles.append(e)
            for idx, (pay, lp) in enumerate(((ptlw, lpw5), (ptlh, lph5))):
                e = pe.tile([ROWS, NT], f32, name=f"e{idx + 2}")
                nc.vector.tensor_tensor(out=e[:], in0=pay, in1=lp[:], op=OP.subtract)
                e_tiles.append(e)

            l1u = pe.tile([ROWS, NT], f32, name="l1u")
            nc.vector.memset(l1u[:], 0.0)
            for idx, e in enumerate(e_tiles):
                ae = wp.tile([ROWS, NT], f32, name=f"ae{idx}", tag="ae")
                nc.scalar.activation(out=ae[:], in_=e[:], func=AF.Abs)
                m = wp.tile([ROWS, NT], f32, name=f"m{idx}", tag="m")
                nc.vector.tensor_scalar(
                    out=m[:], in0=ae[:], scalar1=1.0, scalar2=None, op0=OP.min
                )
                pq = wp.tile([ROWS, NT], f32, name=f"pq{idx}", tag="pq")
                nc.vector.tensor_tensor(out=pq[:], in0=m[:], in1=ae[:], op=OP.mult)
                rr = wp.tile([ROWS, NT], f32, name=f"rr{idx}", tag="rr")
                nc.vector.tensor_scalar(
                    out=rr[:],
                    in0=ae[:],
                    scalar1=1.0,
                    scalar2=0.0,
                    op0=OP.subtract,
                    op1=OP.max,
                )
                nc.vector.tensor_tensor(out=pq[:], in0=pq[:], in1=rr[:], op=OP.add)
                nc.vector.tensor_tensor(out=l1u[:], in0=l1u[:], in1=pq[:], op=OP.add)

            term = pe.tile([ROWS, NT], f32, name="term")
            nc.vector.tensor_tensor(out=term[:], in0=s_sb[:], in1=xf[:], op=OP.mult)
            nc.vector.tensor_tensor(out=term[:], in0=term[:], in1=l1u[:], op=OP.mult)
            pack3 = pe.tile([ROWS, 3], f32, name="pack3")
            nc.vector.tensor_reduce(
                out=pack3[:, 0:1], in_=term[:], axis=mybir.AxisListType.X, op=OP.add
            )
            nc.vector.tensor_reduce(
                out=pack3[:, 1:2], in_=xf[:], axis=mybir.AxisListType.X, op=OP.add
            )
            nc.vector.tensor_reduce(
                out=pack3[:, 2:3], in_=s_sb[:], axis=mybir.AxisListType.X, op=OP.add
            )
            sums_ps = pp.tile([1, 3], f32, name="sums_ps", tag="ps")
            nc.tensor.matmul(
                out=sums_ps[:], lhsT=ones128[:], rhs=pack3[:], start=True, stop=True
            )
            sums_sb = pe.tile([1, 3], f32, name="sums_sb")
            nc.scalar.copy(out=sums_sb[:], in_=sums_ps[:])

            # ---------------- Phase 4: best-prior corrections ----------------
            dnum_g = []
            dden_g = []
            for g in range(2):
                od = pe.tile([ROWS, 1], f32, name=f"od{g}")
                nc.vector.tensor_scalar(
                    out=od[:],
                    in0=p_star[g][:, 0:1],
                    scalar1=POFFb[:],
                    scalar2=None,
                    op0=OP.subtract,
                )
                o1 = wp.tile([ROWS, 1], f32, name=f"o1{g}", tag="o1")
                nc.vector.tensor_scalar(
                    out=o1[:], in0=od[:], scalar1=0.0, scalar2=None, op0=OP.is_ge
                )
                o2 = wp.tile([ROWS, 1], f32, name=f"o2{g}", tag="o2")
                nc.vector.tensor_scalar(
                    out=o2[:], in0=od[:], scalar1=float(PL), scalar2=None, op0=OP.is_lt
                )
                ownk = pe.tile([ROWS, 1], f32, name=f"ownk{g}")
                nc.vector.tensor_tensor(out=ownk[:], in0=o1[:], in1=o2[:], op=OP.mult)
                nc.vector.tensor_tensor(
                    out=ownk[:], in0=ownk[:], in1=keep[g][:], op=OP.mult
                )
                phf = pe.tile([ROWS, 1], f32, name=f"phf{g}")
                nc.vector.tensor_scalar(
                    out=phf[:],
                    in0=od[:],
                    scalar1=0.0,
                    scalar2=float(PL - 1),
                    op0=OP.max,
                    op1=OP.min,
                )
                phu = pe.tile([ROWS, 1], mybir.dt.uint32, name=f"phu{g}")
                nc.vector.tensor_copy(out=phu[:], in_=phf[:])
                rhat_u = pe.tile([ROWS, 1], mybir.dt.uint32, name=f"rhatu{g}")
                nc.vector.tensor_scalar(
                    out=rhat_u[:],
                    in0=phu[:],
                    scalar1=7,
                    scalar2=None,
                    op0=OP.logical_shift_right,
                )
                jhat_u = pe.tile([ROWS, 1], mybir.dt.uint32, name=f"jhatu{g}")
                nc.vector.tensor_scalar(
                    out=jhat_u[:],
                    in0=phu[:],
                    scalar1=127,
                    scalar2=None,
                    op0=OP.bitwise_and,
                )
                rhat_f = pe.tile([ROWS, 1], f32, name=f"rhatf{g}")
                nc.vector.tensor_copy(out=rhat_f[:], in_=rhat_u[:])
                jhat_f = pe.tile([ROWS, 1], f32, name=f"jhatf{g}")
                nc.vector.tensor_copy(out=jhat_f[:], in_=jhat_u[:])

                # gather prior raw data from DRAM
                glocst = pe.tile([ROWS, 2], f32, name=f"glocst{g}")
                nc.gpsimd.indirect_dma_start(
                    out=glocst[:],
                    out_offset=None,
                    in_=locs_ext[:],
                    in_offset=bass.IndirectOffsetOnAxis(ap=phu[:, 0:1], axis=0),
                )
                glocs = pe.tile([ROWS, 2], f32, name=f"glocs{g}")
                nc.scalar.copy(out=glocs[:], in_=glocst[:])
                gparamst = pe.tile([ROWS, 3], f32, name=f"gparamst{g}")
                nc.gpsimd.indirect_dma_start(
                    out=gparamst[:],
                    out_offset=None,
                    in_=params_ext[:],
                    in_offset=bass.IndirectOffsetOnAxis(ap=phu[:, 0:1], axis=0),
                )
                gparams = pe.tile([ROWS, 3], f32, name=f"gparams{g}")
                nc.scalar.copy(out=gparams[:], in_=gparamst[:])

                # per-truth payload columns: reuse the PAY_g tables
                tcols = [PAYg[g][:, c : c + 1] for c in range(4)]

                gcx = glocs[:, 0:1]
                gcy = glocs[:, 1:2]
                gw = gparams[:, 0:1]
                gh = gparams[:, 1:2]
                rgw = pe.tile([ROWS, 1], f32, name=f"rgw{g}")
                nc.vector.reciprocal(out=rgw[:], in_=gw)
                rgh = pe.tile([ROWS, 1], f32, name=f"rgh{g}")
                nc.vector.reciprocal(out=rgh[:], in_=gh)
                lgw5 = pe.tile([ROWS, 1], f32, name=f"lgw5{g}")
                nc.scalar.activation(out=lgw5[:], in_=gw, func=AF.Ln)
                nc.vector.tensor_scalar(
                    out=lgw5[:], in0=lgw5[:], scalar1=5.0, scalar2=None, op0=OP.mult
                )
                lgh5 = pe.tile([ROWS, 1], f32, name=f"lgh5{g}")
                nc.scalar.activation(out=lgh5[:], in_=gh, func=AF.Ln)
                nc.vector.tensor_scalar(
                    out=lgh5[:], in0=lgh5[:], scalar1=5.0, scalar2=None, op0=OP.mult
                )

                encs = []
                for cc, (tcol, gcen, rg_) in enumerate(
                    ((tcols[0], gcx, rgw), (tcols[1], gcy, rgh))
                ):
                    gx10 = wp.tile([ROWS, 1], f32, name=f"gx10_{g}_{cc}", tag="gx10")
                    nc.vector.tensor_scalar(
                        out=gx10[:], in0=gcen, scalar1=10.0, scalar2=None, op0=OP.mult
                    )
                    en = pe.tile([ROWS, 1], f32, name=f"en{g}_{cc}")
                    nc.vector.tensor_tensor(
                        out=en[:], in0=tcol, in1=gx10[:], op=OP.subtract
                    )
                    nc.vector.tensor_tensor(out=en[:], in0=en[:], in1=rg_[:], op=OP.mult)
                    encs.append(en)
                for cc, (tcol, lg_) in enumerate(
                    ((tcols[2], lgw5), (tcols[3], lgh5))
                ):
                    en = pe.tile([ROWS, 1], f32, name=f"en{g}_{cc + 2}")
                    nc.vector.tensor_tensor(
                        out=en[:], in0=tcol, in1=lg_[:], op=OP.subtract
                    )
                    encs.append(en)

                l1n = pe.tile([ROWS, 1], f32, name=f"l1n{g}")
                nc.vector.memset(l1n[:], 0.0)
                for cc, en in enumerate(encs):
                    ae = wp.tile([ROWS, 1], f32, name=f"nae{g}_{cc}", tag="nae")
                    nc.scalar.activation(out=ae[:], in_=en[:], func=AF.Abs)
                    m = wp.tile([ROWS, 1], f32, name=f"nm{g}_{cc}", tag="nm")
                    nc.vector.tensor_scalar(
                        out=m[:], in0=ae[:], scalar1=1.0, scalar2=None, op0=OP.min
                    )
                    pq = wp.tile([ROWS, 1], f32, name=f"npq{g}_{cc}", tag="npq")
                    nc.vector.tensor_tensor(out=pq[:], in0=m[:], in1=ae[:], op=OP.mult)
                    rr = wp.tile([ROWS, 1], f32, name=f"nrr{g}_{cc}", tag="nrr")
                    nc.vector.tensor_scalar(
                        out=rr[:],
                        in0=ae[:],
                        scalar1=1.0,
                        scalar2=0.0,
                        op0=OP.subtract,
                        op1=OP.max,
                    )
                    nc.vector.tensor_tensor(out=pq[:], in0=pq[:], in1=rr[:], op=OP.add)
                    nc.vector.tensor_tensor(out=l1n[:], in0=l1n[:], in1=pq[:], op=OP.add)

                # sandwich gathers of l1u, xf, s at (rhat, jhat)
                rhb_ps = pp.tile([ROWS, ROWS], f32, name=f"rhb_ps{g}", tag="ps")
                nc.tensor.transpose(
                    out=rhb_ps[:],
                    in_=rhat_f[:, 0:1].to_broadcast([ROWS, ROWS]),
                    identity=ident[:],
                )
                rhb = pe.tile([ROWS, ROWS], f32, name=f"rhb{g}")
                nc.scalar.copy(out=rhb[:], in_=rhb_ps[:])
                E_row = pe.tile([ROWS, ROWS], f32, name=f"E_row{g}")
                nc.vector.tensor_scalar(
                    out=E_row[:],
                    in0=rhb[:],
                    scalar1=ridf[:],
                    scalar2=None,
                    op0=OP.is_equal,
                )
                E_colT = pe.tile([ROWS, ROWS], f32, name=f"E_colT{g}")
                nc.vector.tensor_scalar(
                    out=E_colT[:],
                    in0=IOTA_TF[:, 0:ROWS],
                    scalar1=jhat_f[:],
                    scalar2=None,
                    op0=OP.is_equal,
                )

                gath = []
                for ai, arr in enumerate((l1u, xf, s_sb)):
                    B_ps = pp.tile([ROWS, ROWS], f32, name=f"B_ps{g}_{ai}", tag="ps")
                    nc.tensor.matmul(
                        out=B_ps[:], lhsT=arr[:], rhs=E_row[:], start=True, stop=True
                    )
                    Bs = wp.tile([ROWS, ROWS], f32, name=f"Bs{g}_{ai}", tag="Bs")
                    nc.scalar.copy(out=Bs[:], in_=B_ps[:])
                    BT_ps = pp.tile(
                        [ROWS, ROWS], f32, name=f"BT_ps{g}_{ai}", tag="ps"
                    )
                    nc.tensor.transpose(out=BT_ps[:], in_=Bs[:], identity=ident[:])
                    BTs = wp.tile([ROWS, ROWS], f32, name=f"BTs{g}_{ai}", tag="BTs")
                    nc.scalar.copy(out=BTs[:], in_=BT_ps[:])
                    acc = pe.tile([ROWS, 1], f32, name=f"gath{g}_{ai}")
                    trash4 = wp.tile(
                        [ROWS, ROWS], f32, name=f"trash4{g}_{ai}", tag="trash4"
                    )
                    nc.vector.tensor_tensor(
                        out=trash4[:], in0=BTs[:], in1=E_colT[:], op=OP.mult
                    )
                    nc.vector.tensor_reduce(
                        out=acc[:], in_=trash4[:], axis=mybir.AxisListType.X, op=OP.add
                    )
                    gath.append(acc)
                l1_at, xf_at, s_at = gath

                w1 = wp.tile([ROWS, 1], f32, name=f"w1{g}", tag="w1")
                nc.vector.tensor_scalar(
                    out=w1[:], in0=l1n[:], scalar1=K_VAL, scalar2=None, op0=OP.mult
                )
                w2 = wp.tile([ROWS, 1], f32, name=f"w2{g}", tag="w2")
                nc.vector.tensor_tensor(out=w2[:], in0=xf_at[:], in1=l1_at[:], op=OP.mult)
                nc.vector.tensor_tensor(out=w1[:], in0=w1[:], in1=w2[:], op=OP.subtract)
                nc.vector.tensor_tensor(out=w1[:], in0=w1[:], in1=s_at[:], op=OP.mult)
                dn = pe.tile([ROWS, 1], f32, name=f"dn{g}")
                nc.vector.tensor_tensor(out=dn[:], in0=w1[:], in1=ownk[:], op=OP.mult)
                dd = pe.tile([ROWS, 1], f32, name=f"dd{g}")
                nc.vector.tensor_scalar(
                    out=dd[:],
                    in0=xf_at[:],
                    scalar1=-1.0,
                    scalar2=K_VAL,
                    op0=OP.mult,
                    op1=OP.add,
                )
                nc.vector.tensor_tensor(out=dd[:], in0=dd[:], in1=ownk[:], op=OP.mult)
                dnum_g.append(dn)
                dden_g.append(dd)

            pack2 = pe.tile([ROWS, 2], f32, name="pack2")
            nc.vector.tensor_tensor(
                out=pack2[:, 0:1], in0=dnum_g[0][:], in1=dnum_g[1][:], op=OP.add
            )
            nc.vector.tensor_tensor(
                out=pack2[:, 1:2], in0=dden_g[0][:], in1=dden_g[1][:], op=OP.add
            )
            sums2_ps = pp.tile([1, 2], f32, name="sums2_ps", tag="ps")
            nc.tensor.matmul(
                out=sums2_ps[:], lhsT=ones128[:], rhs=pack2[:], start=True, stop=True
            )
            sums2_sb = pe.tile([1, 2], f32, name="sums2_sb")
            nc.scalar.copy(out=sums2_sb[:], in_=sums2_ps[:])

            # ---------------- Phase 5: final AllReduce + loss ----------------
            asm = pe.tile([1, 8], f32, name="asm")
            nc.vector.memset(asm[:], 0.0)
            nc.vector.tensor_copy(out=asm[:, 0:3], in_=sums_sb[:])
            nc.vector.tensor_copy(out=asm[:, 3:5], in_=sums2_sb[:])
            ar_in = dp.tile([1, 8], f32, name="ar_in")
            nc.sync.dma_start(out=ar_in[:], in_=asm[:])
            ar_out = dp.tile([1, 8], f32, name="ar_out", addr_space="Shared")
            nc.gpsimd.collective_compute(
                "AllReduce",
                mybir.AluOpType.add,
                ins=[ar_in[:]],
                outs=[ar_out[:]],
                replica_groups=RG,
            )
            gsumt = pe.tile([1, 8], f32, name="gsumt")
            nc.sync.dma_start(out=gsumt[:], in_=ar_out[:])
            gsum = pe.tile([1, 8], f32, name="gsum")
            nc.scalar.copy(out=gsum[:], in_=gsumt[:])

            num = pe.tile([1, 1], f32, name="num")
            nc.vector.tensor_tensor(
                out=num[:], in0=gsum[:, 0:1], in1=gsum[:, 3:4], op=OP.add
            )
            nc.vector.tensor_scalar(
                out=num[:], in0=num[:], scalar1=0.5, scalar2=None, op0=OP.mult
            )
            nc.vector.tensor_tensor(
                out=num[:], in0=num[:], in1=gsum[:, 2:3], op=OP.add
            )  # + BETA * sum(sigmoid)
            den = pe.tile([1, 1], f32, name="den")
            nc.vector.tensor_tensor(
                out=den[:], in0=gsum[:, 1:2], in1=gsum[:, 4:5], op=OP.add
            )
            rden = pe.tile([1, 1], f32, name="rden")
            nc.vector.reciprocal(out=rden[:], in_=den[:])
            loss = pe.tile([1, 1], f32, name="loss")
            nc.vector.tensor_tensor(out=loss[:], in0=num[:], in1=rden[:], op=OP.mult)
            nc.sync.dma_start(out=out_ext[:], in_=loss[:])

    _split_waits(nc)
    return nc


def _split_waits(nc):
    """This toolchain's codegen accepts only one embedded sem-wait per
    instruction; hoist extra waits into standalone EventSemaphore
    instructions on the same engine (same blocking semantics)."""
    import orjson

    import copy as _copy

    d = orjson.loads(nc.to_json_bytes())
    ctr = 0
    for fn in d.get("functions", []):
        for bb in fn.get("blocks", []):
            out = []
            for ins in bb.get("instructions", []):
                if (
                    ins.get("opcode") == "ISA"
                    and ins.get("op_name") == "EVENT_SEMAPHORE_RANGE_CLEAR"
                ):
                    # codegen rejects clear ranges wider than 16 sems; split.
                    first, last = ins["instr"][13], ins["instr"][14]
                    if last - first + 1 > 16:
                        lo = first
                        while lo <= last:
                            hi = min(lo + 15, last)
                            ctr += 1
                            part = _copy.deepcopy(ins)
                            part["name"] = f"{ins['name']}_rc{ctr}"
                            part["instr"] = list(ins["instr"])
                            part["instr"][13] = lo
                            part["instr"][14] = hi
                            if lo != first:
                                part["sync_info"] = {"on_wait": [], "on_update": []}
                            out.append(part)
                            lo = hi + 1
                        continue
                si = ins.get("sync_info")
                ow = (si or {}).get("on_wait") or []
                if si and len(ow) > 1 and "engine" in ins:
                    for w in ow[:-1]:
                        ctr += 1
                        ev = {
                            "engine": ins["engine"],
                            "ins": [],
                            "outs": [],
                            "name": f"antsplit_{ctr}",
                            "opcode": "EventSemaphore",
                            "sync_info": {"on_wait": [w], "on_update": []},
                        }
                        if "debug" in ins:
                            ev["debug"] = ins["debug"]
                        out.append(ev)
                    si["on_wait"] = [ow[-1]]
                out.append(ins)
            bb["instructions"] = out
    blob = orjson.dumps(d)
    nc.to_json_bytes = lambda: blob
    return nc


def kernel(**inputs):
    locs = np.ascontiguousarray(np.asarray(inputs["locs"], dtype=np.float32))
    params = np.ascontiguousarray(np.asarray(inputs["params"], dtype=np.float32))
    truths = np.ascontiguousarray(np.asarray(inputs["truths"], dtype=np.float32))

    if "nc" not in _CACHE:
        _CACHE["nc"] = _build()
    nc = _CACHE["nc"]

    in_maps = []
    for c in range(NCORES):
        in_maps.append(
            {
                "locs": locs[c * PL : (c + 1) * PL],
                "params": params[c * PL : (c + 1) * PL],
                "truths": truths,
                "poff": np.array([[c * PL]], dtype=np.float32),
            }
        )

    from concourse.bass_utils import run_bass_kernel_spmd

    res = run_bass_kernel_spmd(nc, in_maps, core_ids=list(range(NCORES)))
    out = np.asarray(res.results[0]["out"], dtype=np.float32)
    return out.reshape(())


if __name__ == "__main__":
    sys.path.insert(0, "/root/problem")
    import reference

    inputs = {k: np.asarray(v) for k, v in reference.setup_inputs().items()}
    expected = np.asarray(reference.reference(**inputs))
    actual = kernel(**inputs)
    rel = abs(float(actual) - float(expected)) / max(abs(float(expected)), 1e-12)
    print("expected:", expected, "actual:", actual, "rel_err:", rel)
